# revision 8
# baseline (speedup 1.0000x reference)
"""GQA kernel for 8 trn2 NeuronCores.

Sharding: tensor-parallel over heads. Core c owns KV head c and Q heads
4c..4c+3 (q-dim cols 256c:256c+256 of Wq, col 64c:64c+64 of Wk/Wv, rows
256c:256c+256 of Wo). Each core computes a partial output [B,S,E]
(its ctx slice @ its Wo row-slice); host sums the 8 partials.

Device algorithm (per core, per batch) — v3 (cost-model driven):
  A1. Q.T pair tiles [128, S] (heads 2p/2p+1 stacked), scale folded into
      host-side Wq.
  A2. K/V projections with X-as-lhsT: out K/V natural [t,64] (N=64
      matmuls — half the PE rows of the N=512 orientation). V lands
      directly in V_aug [kv,65] layout (ones col for the softmax
      denominator); K is DMA-transposed into K.T [64, S] + dup.
  B.  per (p, jq, head): scores S.T [kv,q] (as before); exp on ScalarE
      -> P.T bf16; ctx via P-as-lhsT: out ctx[q, 65] (col 64 = sum P =
      denominator), N=65 matmuls accumulated over 16 kv chunks.
      Normalize: DVE recip [128,1] + tensor_scalar_mul -> ctx bf16;
      PE-transpose (identity matmul) pairs -> ctx.T tiles for C.
  C.  out_partial[t,:] = ctx.T.T @ Wo_c; PSUM evacuated on GpSimd,
      one [128, 2048] DMA per t-chunk.

Emission is software-pipelined at sub-phase granularity: B is
ScalarE(exp)-bound, so A-phase (next batch) / C-phase (prev batch) PE
units are interleaved between B units to keep the in-order PE queue fed.
"""

import numpy as np
import ml_dtypes

B = 2
S = 2048
E = 2048
HD = 64          # head dim
HPC = 4          # q heads per core
NP = 2           # head pairs per core
QD = HPC * HD    # 256 per-core q dims
NCORES = 8
EC = E // 128    # 16 contraction chunks
NJQ = S // 512   # 4 q-chunks of 512
NKV = S // 128   # 16 kv chunks of 128
BF16 = ml_dtypes.bfloat16

_cache = {}


def _interleave(main_gen, side_gen, ratio):
    """Drive main_gen, pulling one side_gen unit every `ratio` main units."""
    n = 0
    for _ in main_gen:
        n += 1
        if side_gen is not None and n % ratio == 0:
            next(side_gen, None)
    if side_gen is not None:
        for _ in side_gen:
            pass


def _build():
    from contextlib import ExitStack
    from concourse import bacc, tile, masks
    import concourse.mybir as mybir

    bf16 = mybir.dt.bfloat16
    f32 = mybir.dt.float32
    EXP = mybir.ActivationFunctionType.Exp

    nc = bacc.Bacc(
        "TRN2", target_bir_lowering=False, debug=False, num_devices=NCORES)
    qT_d = nc.declare_dram_parameter("qT", [B, E, S], bf16, isOutput=False)
    kT_d = nc.declare_dram_parameter("kT", [B, E, S], bf16, isOutput=False)
    vT_d = nc.declare_dram_parameter("vT", [B, E, S], bf16, isOutput=False)
    wq_d = nc.declare_dram_parameter("wq", [E, QD], bf16, isOutput=False)
    wk_d = nc.declare_dram_parameter("wk", [E, HD], bf16, isOutput=False)
    wv_d = nc.declare_dram_parameter("wv", [E, HD], bf16, isOutput=False)
    wo_d = nc.declare_dram_parameter("wo", [QD, E], bf16, isOutput=False)
    out_d = nc.declare_dram_parameter("out", [B, S, E], bf16, isOutput=True)

    with ExitStack() as ctx:
        tc = ctx.enter_context(tile.TileContext(nc))
        # ---- pools ----
        wpool = ctx.enter_context(tc.tile_pool(name="w", bufs=1))
        qin = ctx.enter_context(tc.tile_pool(name="qin", bufs=16))
        kvin = ctx.enter_context(tc.tile_pool(name="kvin", bufs=4))
        qpp = ctx.enter_context(tc.tile_pool(name="qpp", bufs=4))
        kt2p = ctx.enter_context(tc.tile_pool(name="kt2p", bufs=2))
        knp = ctx.enter_context(tc.tile_pool(name="knp", bufs=4))
        vnp = ctx.enter_context(tc.tile_pool(name="vnp", bufs=32))
        ptp = ctx.enter_context(tc.tile_pool(name="ptp", bufs=20))
        cnp = ctx.enter_context(tc.tile_pool(name="cnp", bufs=4))
        rcp = ctx.enter_context(tc.tile_pool(name="rcp", bufs=8))
        ctxTp = ctx.enter_context(tc.tile_pool(name="ctxTp", bufs=4))
        ostp = ctx.enter_context(tc.tile_pool(name="ostp", bufs=2))
        # PSUM: psc 2x[128,1024]f32 (4 banks: scores + A2 accs),
        # psq 2x[128,512]f32 (2 banks: A1/C accs + transpose outs),
        # psv 2x[128,512]f32 (2 banks: ctx accs) = 8 banks total.
        psc = ctx.enter_context(tc.tile_pool(name="psc", bufs=2, space="PSUM"))
        psq = ctx.enter_context(tc.tile_pool(name="psq", bufs=2, space="PSUM"))
        psv = ctx.enter_context(tc.tile_pool(name="psv", bufs=2, space="PSUM"))

        # ---- weights (loaded once) ----
        wq_sb = wpool.tile([128, EC, QD], bf16)
        nc.sync.dma_start(wq_sb[:], wq_d.rearrange("(c p) m -> p c m", p=128))
        wk_sb = wpool.tile([128, EC, HD], bf16)
        nc.sync.dma_start(wk_sb[:], wk_d.rearrange("(c p) m -> p c m", p=128))
        wv_sb = wpool.tile([128, EC, HD], bf16)
        nc.sync.dma_start(wv_sb[:], wv_d.rearrange("(c p) m -> p c m", p=128))
        wo_sb = wpool.tile([128, 2, E], bf16)
        nc.sync.dma_start(wo_sb[:], wo_d.rearrange("(c p) e -> p c e", p=128))
        ident = wpool.tile([128, 128], bf16)
        masks.make_identity(nc, ident[:])

        # per-batch state handles
        qp_sb = [[None] * NP for _ in range(B)]
        kt2_sb = [None] * B
        vn_tiles = [[None] * NKV for _ in range(B)]
        ctxT_sb = [[None] * NP for _ in range(B)]

        def phase_A1(b):
            # ---------- A1: Q.T as pair tiles [128, S] ----------
            qtiles = []
            for e in range(EC):
                qt = qin.tile([128, S], bf16, tag="qt", name="qt")
                nc.sync.dma_start(qt[:], qT_d[b, e * 128:(e + 1) * 128, :])
                qtiles.append(qt)
            for m in range(NP):
                qp_sb[b][m] = qpp.tile([128, S], bf16, tag="qp", name="qp")
            for m in range(NP):
                for t in range(NJQ):
                    acc = psq.tile([128, 512], f32, tag="qa", name="acc")
                    for e in range(EC):
                        nc.tensor.matmul(
                            acc[:], lhsT=wq_sb[:, e, m * 128:(m + 1) * 128],
                            rhs=qtiles[e][:, t * 512:(t + 1) * 512],
                            start=(e == 0), stop=(e == EC - 1))
                    nc.vector.tensor_copy(
                        qp_sb[b][m][:, t * 512:(t + 1) * 512], acc[:])
                    yield

        def phase_A2(b):
            # ---------- A2: K/V natural via X-as-lhsT (N=64) ----------
            acc_k = psc.tile([128, 1024], f32, tag="sc", name="acc_k")
            acc_v = psc.tile([128, 1024], f32, tag="sc", name="acc_v")
            for e in range(EC):
                kt_in = kvin.tile([128, S], bf16, tag="kv", name="kt_in")
                nc.sync.dma_start(kt_in[:], kT_d[b, e * 128:(e + 1) * 128, :])
                vt_in = kvin.tile([128, S], bf16, tag="kv", name="vt_in")
                nc.sync.dma_start(vt_in[:], vT_d[b, e * 128:(e + 1) * 128, :])
                # one accumulation group per 2KB PSUM bank (zero region):
                # start only on the bank's first matmul (first-touch-zero
                # covers the other tc slices), stop on its last.
                for tc_ in range(NKV):
                    st = (e == 0) and (tc_ % 8 == 0)
                    sp = (e == EC - 1) and (tc_ % 8 == 7)
                    nc.tensor.matmul(
                        acc_k[:, tc_ * 64:(tc_ + 1) * 64],
                        lhsT=kt_in[:, tc_ * 128:(tc_ + 1) * 128],
                        rhs=wk_sb[:, e, :],
                        start=st, stop=sp)
                    nc.tensor.matmul(
                        acc_v[:, tc_ * 64:(tc_ + 1) * 64],
                        lhsT=vt_in[:, tc_ * 128:(tc_ + 1) * 128],
                        rhs=wv_sb[:, e, :],
                        start=st, stop=sp)
                if e % 4 == 3:
                    yield
            kt2_sb[b] = kt2p.tile([128, S], bf16, tag="kt2", name="kt2")
            for tc_ in range(NKV):
                # kn holds the K chunk twice side by side; the [128,128]
                # xbar transpose then yields K.T in rows 0:64 AND the
                # row-64:128 duplicate (for head-o score matmuls) in one
                # shot. (xbar needs free_dim % 128 == 0 anyway.)
                kn = knp.tile([128, 2 * HD], bf16, tag="kn", name="kn")
                nc.vector.tensor_copy(
                    kn[:, 0:HD], acc_k[:, tc_ * 64:(tc_ + 1) * 64])
                nc.vector.tensor_copy(
                    kn[:, HD:2 * HD], acc_k[:, tc_ * 64:(tc_ + 1) * 64])
                nc.sync.dma_start_transpose(
                    out=kt2_sb[b][:, tc_ * 128:(tc_ + 1) * 128], in_=kn[:])
                vn = vnp.tile([128, HD + 1], bf16, tag="vn", name="vn")
                nc.gpsimd.memset(vn[:, HD:HD + 1], 1.0)
                nc.vector.tensor_copy(
                    vn[:, 0:HD], acc_v[:, tc_ * 64:(tc_ + 1) * 64])
                vn_tiles[b][tc_] = vn
                if tc_ % 4 == 3:
                    yield

        def phase_B(b):
            for p in range(NP):
                ctxT_sb[b][p] = ctxTp.tile(
                    [128, S], bf16, tag="ctxT", name="ctxT")
            for p in range(NP):
                for jq in range(NJQ):
                    for hp in range(2):
                        rh = hp * HD
                        pts = []
                        for g in range(NKV // 2):
                            sc = psc.tile([128, 1024], f32, tag="sc",
                                          name="sc")
                            for ki in range(2):
                                kv = g * 2 + ki
                                nc.tensor.matmul(
                                    sc[:, ki * 512:(ki + 1) * 512],
                                    lhsT=kt2_sb[b][rh:rh + HD,
                                                   kv * 128:(kv + 1) * 128],
                                    rhs=qp_sb[b][p][rh:rh + HD,
                                                    jq * 512:(jq + 1) * 512],
                                    start=True, stop=True)
                            pt = ptp.tile([128, 1024], bf16, tag="pt",
                                          name="pt")
                            nc.scalar.activation(pt[:], sc[:], EXP)
                            pts.append(pt)
                            yield
                        # ctx: P-as-lhsT, out [q,65] (col 64 = denominator)
                        acc = psv.tile([128, 512], f32, tag="cx", name="cacc")
                        # single accumulation group for the whole bank: the
                        # 4 qi sub-chains rely on first-touch-zero semantics
                        for qi in range(4):
                            for kv in range(NKV):
                                g, ki = kv // 2, kv % 2
                                nc.tensor.matmul(
                                    acc[:, qi * 128:qi * 128 + HD + 1],
                                    lhsT=pts[g][:, ki * 512 + qi * 128:
                                                ki * 512 + (qi + 1) * 128],
                                    rhs=vn_tiles[b][kv][:, 0:HD + 1],
                                    start=(qi == 0 and kv == 0),
                                    stop=(qi == 3 and kv == NKV - 1))
                        yield
                        # normalize + transpose into ctx.T
                        for qh in range(2):
                            cn = cnp.tile([128, 128], bf16, tag="cn",
                                          name="cn")
                            for q2 in range(2):
                                qi = qh * 2 + q2
                                rc = rcp.tile([128, 1], f32, tag="rc",
                                              name="rc")
                                nc.vector.reciprocal(
                                    rc[:],
                                    acc[:, qi * 128 + HD:qi * 128 + HD + 1])
                                nc.vector.tensor_scalar_mul(
                                    cn[:, q2 * 64:(q2 + 1) * 64],
                                    acc[:, qi * 128:qi * 128 + HD], rc[:])
                            pst = psq.tile([128, 128], bf16, tag="qa",
                                           name="pst")
                            nc.tensor.transpose(pst[:], cn[:], ident[:])
                            for q2 in range(2):
                                qi = qh * 2 + q2
                                nc.vector.tensor_copy(
                                    ctxT_sb[b][p][rh:rh + HD,
                                                  jq * 512 + qi * 128:
                                                  jq * 512 + (qi + 1) * 128],
                                    pst[q2 * 64:q2 * 64 + HD, :])
                            yield

        def phase_C(b):
            for t in range(S // 128):
                ost = ostp.tile([128, E], bf16, tag="ost", name="ost")
                for ec in range(E // 512):
                    acc = psq.tile([128, 512], f32, tag="qa", name="oacc")
                    for kc in range(2):
                        nc.tensor.matmul(
                            acc[:],
                            lhsT=ctxT_sb[b][kc][:, t * 128:(t + 1) * 128],
                            rhs=wo_sb[:, kc, ec * 512:(ec + 1) * 512],
                            start=(kc == 0), stop=(kc == 1))
                    # gpsimd can't read PSUM; split evacuation DVE/ScalarE
                    if ec % 2 == 0:
                        nc.vector.tensor_copy(
                            ost[:, ec * 512:(ec + 1) * 512], acc[:])
                    else:
                        nc.scalar.copy(
                            ost[:, ec * 512:(ec + 1) * 512], acc[:])
                    yield
                nc.sync.dma_start(
                    out_d[b, t * 128:(t + 1) * 128, :], ost[:])

        # software-pipelined emission:
        #   A1(0) A2(0) | B(0)+A1(1) | A2(1) | B(1)+C(0) | C(1)
        # (A2 is never interleaved with B: its accs share the "sc" PSUM
        # tag with B's score tiles and are held across the whole e-loop,
        # which would head-of-line-block the in-order PE queue.)
        for _ in phase_A1(0):
            pass
        for _ in phase_A2(0):
            pass
        _interleave(phase_B(0), phase_A1(1), 20)
        for _ in phase_A2(1):
            pass
        _interleave(phase_B(1), phase_C(0), 3)
        for _ in phase_C(1):
            pass
    nc.compile()
    return nc


def _get_nc():
    if "nc" not in _cache:
        _cache["nc"] = _build()
    return _cache["nc"]


def kernel(query, key, value, Wq, Wk, Wv, Wo, _trace=False):
    from concourse.bass_utils import run_bass_kernel_spmd

    def t_bf16(x):
        return np.ascontiguousarray(
            np.asarray(x, np.float32).astype(BF16).transpose(0, 2, 1))

    qT = t_bf16(query)
    kT = t_bf16(key)
    vT = t_bf16(value)
    # fold the 1/sqrt(HD) score scale into Wq host-side
    Wq = (np.asarray(Wq, np.float32) * 0.125).astype(BF16)
    Wk = np.asarray(Wk, np.float32).astype(BF16)
    Wv = np.asarray(Wv, np.float32).astype(BF16)
    Wo = np.asarray(Wo, np.float32).astype(BF16)

    in_maps = []
    for c in range(NCORES):
        in_maps.append({
            "qT": qT, "kT": kT, "vT": vT,
            "wq": np.ascontiguousarray(Wq[:, c * QD:(c + 1) * QD]),
            "wk": np.ascontiguousarray(Wk[:, c * HD:(c + 1) * HD]),
            "wv": np.ascontiguousarray(Wv[:, c * HD:(c + 1) * HD]),
            "wo": np.ascontiguousarray(Wo[c * QD:(c + 1) * QD, :]),
        })

    nc = _get_nc()
    res = run_bass_kernel_spmd(nc, in_maps, list(range(NCORES)), trace=_trace)
    out = res.results[0]["out"].astype(np.float32)
    for c in range(1, NCORES):
        out += res.results[c]["out"].astype(np.float32)
    if _trace:
        _cache["last_exec_time_ns"] = res.exec_time_ns
        _cache["last_results"] = res
    return out
